# revision 1
# baseline (speedup 1.0000x reference)
"""BrainRNN forward pass on 8 TRN2 NeuronCores (Bass/Tile, SPMD).

Strategy (tensor-parallel over output neurons, fp32 exact):
  - Each block's 1024 output neurons are row-sharded 128/core; masks are
    folded into weights on the host (memory-bound: halves HBM traffic).
  - Matmuls run with the (streamed) weights as the MOVING operand and the
    activations as the 64-wide stationary operand: psum[b, m] += sum_k
    actT[k, b] * W.T[k, m].  fp32 moving runs ~2 cyc/row vs ~4x slower
    weights-stationary LDWEIGHTS.
  - Adjacent blocks share one [64, 256] PSUM accumulator (bank budget);
    per-block biases enter via a K=1 "ones x biasrow" matmul that opens
    each accumulator.
  - Chain per block: sigmoid (PSUM->SBUF), PE transpose to [128(m), 64(b)],
    DVE copy, gpsimd bounce DMA, AllGather over 8 cores, 3-way staggered
    unpack on the scalar ring.
  - DMA emission order tracks the chain's deadlines (rec block j due at
    sigmoid_j); matmul emission back-fills AllGather wait windows with
    ready work (later-pair skip contributions) to keep the PE warm.
  - Output block: each core contracts its own 128 rows of cur7 against
    W_out[:, rows].T (N=512), then one ReduceScatter yields each core an
    8-row batch shard of the [64, 512] output.
"""

import numpy as np

N = 8192
W = 1024
L = 8
B = 64
IN = 512
OUT = 512
NCORES = 8
RP = W // NCORES        # 128 rows per core per block

_BUILT = None


def _pack(A):
    """[M, K] -> [128, (K/128)*M] with packed[p, k*M+m] = A[m, k*128+p].

    Chunk kidx is A[:, kidx*128:(kidx+1)*128].T, i.e. [K=128(part), M(free)].
    """
    M, K = A.shape
    nk = K // 128
    return np.ascontiguousarray(
        A.reshape(M, nk, 128).transpose(2, 1, 0).reshape(128, nk * M)
    )


def _interleave(Pa, Pb):
    """Two packed [128, nk*128] -> [128, nk*256] with per-chunk interleave."""
    nk = Pa.shape[1] // 128
    out = np.empty((128, nk, 2, 128), np.float32)
    out[:, :, 0, :] = Pa.reshape(128, nk, 128)
    out[:, :, 1, :] = Pb.reshape(128, nk, 128)
    return np.ascontiguousarray(out.reshape(128, nk * 256))


def _build():
    import concourse.bass as bass
    import concourse.bacc as bacc
    import concourse.mybir as mybir
    import concourse.tile as tile

    fp32 = mybir.dt.float32
    AF = mybir.ActivationFunctionType

    nc = bacc.Bacc(
        "TRN2",
        target_bir_lowering=False,
        debug=False,
        enable_asserts=False,
        num_devices=NCORES,
    )

    t_hT = nc.dram_tensor("hT", [128, 64 * B], fp32, kind="ExternalInput")
    t_xT = nc.dram_tensor("xT", [128, 4 * B], fp32, kind="ExternalInput")
    t_win = nc.dram_tensor("win", [128, 4 * RP], fp32, kind="ExternalInput")
    t_rec = {
        j: nc.dram_tensor(f"rec{j}", [128, 64 * 128], fp32, kind="ExternalInput")
        for j in range(7)
    }
    t_hid = nc.dram_tensor("hid", [7, 128, 8 * RP], fp32, kind="ExternalInput")
    # skip{q}_{c}: pair q=(2q, 2q+1), cur-source block c.  c < 2q: both
    # blocks interleaved [128, 8*256]; c == 2q: later block only [128, 8*128].
    t_skip = {}
    for q in range(1, 4):
        a = 2 * q
        for c in range(a + 1):
            wdt = 8 * 256 if c < a else 8 * 128
            t_skip[(q, c)] = nc.dram_tensor(
                f"skip{q}_{c}", [128, wdt], fp32, kind="ExternalInput"
            )
    t_biasrow = nc.dram_tensor("biasrow", [1, 4 * 256], fp32, kind="ExternalInput")
    t_wout = nc.dram_tensor("wout", [128, 512], fp32, kind="ExternalInput")
    t_boutrow = nc.dram_tensor("boutrow", [1, 512], fp32, kind="ExternalInput")
    t_ones = nc.dram_tensor("ones", [1, B], fp32, kind="ExternalInput")
    t_ident = nc.dram_tensor("ident", [B, B], fp32, kind="ExternalInput")
    t_out = nc.dram_tensor("out", [8, 512], fp32, kind="ExternalOutput")

    rg = [list(range(NCORES))]
    qof = lambda j: j // 2          # pair index of block j
    side = lambda j: j % 2          # column side within pair tile

    with tile.TileContext(nc) as tc:
        with (
            tc.tile_pool(name="const", bufs=1) as constp,
            tc.tile_pool(name="wrec", bufs=3) as wrecp,
            tc.tile_pool(name="whid", bufs=1) as whidp,
            tc.tile_pool(name="wskip", bufs=1) as wskipp,
            tc.tile_pool(name="curs", bufs=1) as curp,
            tc.tile_pool(name="psum", bufs=1, space="PSUM") as psump,
            tc.tile_pool(name="dram", bufs=3, space="DRAM") as dramp,
        ):
            # ---- persistent inputs -------------------------------------
            hT_sb = constp.tile([128, 64 * B], fp32, name="hT_sb", tag="hT")
            nc.sync.dma_start(out=hT_sb, in_=t_hT[:, :])
            xT_sb = constp.tile([128, 4 * B], fp32, name="xT_sb", tag="xT")
            nc.sync.dma_start(out=xT_sb, in_=t_xT[:, :])
            win_sb = constp.tile([128, 4 * RP], fp32, name="win_sb", tag="win")
            nc.sync.dma_start(out=win_sb, in_=t_win[:, :])
            biasrow_sb = constp.tile([1, 4 * 256], fp32, name="biasrow_sb", tag="br")
            nc.sync.dma_start(out=biasrow_sb, in_=t_biasrow[:, :])
            wout_sb = constp.tile([128, 512], fp32, name="wout_sb", tag="wout")
            nc.sync.dma_start(out=wout_sb, in_=t_wout[:, :])
            boutrow_sb = constp.tile([1, 512], fp32, name="boutrow_sb", tag="bo")
            nc.sync.dma_start(out=boutrow_sb, in_=t_boutrow[:, :])
            ones_sb = constp.tile([1, B], fp32, name="ones_sb", tag="ones")
            nc.sync.dma_start(out=ones_sb, in_=t_ones[:, :])
            ident_sb = constp.tile([B, B], fp32, name="ident_sb", tag="ident")
            nc.sync.dma_start(out=ident_sb, in_=t_ident[:, :])

            psA = [
                psump.tile([64, 256], fp32, name=f"psA{q}", tag=f"psA{q}")
                for q in range(4)
            ]
            curT = [None] * 8

            # bias-init: psA[q] = ones.T @ biasrow[q]  (start=True opens group)
            for q in range(4):
                nc.tensor.matmul(
                    psA[q],
                    lhsT=ones_sb[:, :],
                    rhs=biasrow_sb[:, q * 256:(q + 1) * 256],
                    start=True,
                    stop=False,
                )

            def rec_blk(j):
                """Stream block j's rec weights (1MB tiles, 512KB DMAs)."""
                q, s = qof(j), side(j)
                for h in range(4):
                    rt = wrecp.tile([128, 2048], fp32, name=f"rec{j}h{h}", tag="rec")
                    for i in range(2):
                        nc.sync.dma_start(
                            out=rt[:, i * 1024:(i + 1) * 1024],
                            in_=t_rec[j][:, h * 2048 + i * 1024:
                                         h * 2048 + (i + 1) * 1024],
                        )
                    for k in range(16):
                        kg = h * 16 + k
                        nc.tensor.matmul(
                            psA[q][:, s * 128:(s + 1) * 128],
                            lhsT=hT_sb[:, kg * B:(kg + 1) * B],
                            rhs=rt[:, k * 128:(k + 1) * 128],
                            start=False,
                            stop=False,
                        )

            def chain_tail(j):
                """sigmoid -> transpose -> AllGather -> staggered unpack."""
                q, s = qof(j), side(j)
                cp = curp.tile([64, 128], fp32, name=f"cpart{j}", tag="cpart", bufs=2)
                nc.scalar.activation(cp, psA[q][:, s * 128:(s + 1) * 128], AF.Sigmoid)
                pt = psump.tile([128, B], fp32, name=f"pt{j}", tag="pt", bufs=2)
                nc.tensor.transpose(pt, cp, ident_sb[:, :])
                ptsb = curp.tile([128, B], fp32, name=f"ptsb{j}", tag="ptsb", bufs=2)
                nc.vector.tensor_copy(ptsb, pt)
                agin = dramp.tile([128, B], fp32, name=f"agin{j}", tag="agin")
                agout = dramp.tile([W, B], fp32, name=f"agout{j}", tag="agout")
                nc.gpsimd.dma_start(out=agin, in_=ptsb)
                nc.gpsimd.collective_compute(
                    "AllGather",
                    mybir.AluOpType.bypass,
                    replica_groups=rg,
                    ins=[agin.opt()],
                    outs=[agout.opt()],
                )
                dst = curp.tile([128, 8 * B], fp32, name=f"curT{j}", tag="curT",
                                bufs=6)
                # gpsimd just observed the collective's completion sem: it can
                # issue chunks 0-3 with no cross-engine hop; scalar does 4-7
                # in parallel on its own ring.
                for eng, kk in ((nc.gpsimd, 0), (nc.scalar, 4)):
                    eng.dma_start(
                        out=dst[:, kk * B:(kk + 4) * B].rearrange(
                            "p (k b) -> p k b", k=4
                        ),
                        in_=agout[kk * 128:(kk + 4) * 128, :].rearrange(
                            "(k p) b -> p k b", p=128
                        ),
                    )
                curT[j] = dst

            hid_tiles = {}

            def hid_tile(j):
                # resident (distinct tag): allocation can never stall the ring
                ht = whidp.tile([128, 8 * RP], fp32, name=f"hid{j}", tag=f"hid{j}")
                nc.sync.dma_start(out=ht, in_=t_hid[j - 1, :, :])
                hid_tiles[j] = ht

            def hid_mms(j):
                """cur_{j-1} @ W_hid[j-1].T into block j's psum columns."""
                q, s = qof(j), side(j)
                if j not in hid_tiles:
                    hid_tile(j)
                ht = hid_tiles[j]
                for kk in range(8):
                    nc.tensor.matmul(
                        psA[q][:, s * 128:(s + 1) * 128],
                        lhsT=curT[j - 1][:, kk * B:(kk + 1) * B],
                        rhs=ht[:, kk * RP:(kk + 1) * RP],
                        start=False,
                        stop=(s == 1 and kk == 7),
                    )

            skip_tiles = {}

            def skip_tile(q, c):
                """Allocate + DMA pair q's skip weights sourced from cur_c."""
                a = 2 * q
                wide = 256 if c < a else 128
                st = wskipp.tile(
                    [128, 8 * wide], fp32, name=f"skip{q}_{c}t", tag=f"skip{q}_{c}"
                )
                half = 4 * wide
                for i in range(2):
                    nc.sync.dma_start(
                        out=st[:, i * half:(i + 1) * half],
                        in_=t_skip[(q, c)][:, i * half:(i + 1) * half],
                    )
                skip_tiles[(q, c)] = st

            def skip_one(q, c, part=None):
                """MMs for pair q's skip from cur_c.  part: None=all columns,
                'lo'=first block's 128 cols, 'hi'=second block's 128 cols."""
                a = 2 * q
                wide = 256 if c < a else 128
                if (q, c) not in skip_tiles:
                    skip_tile(q, c)
                st = skip_tiles[(q, c)]
                off = 0 if c < a else 128
                rlo, rwide = 0, wide
                if part == "lo" and wide == 256:
                    rlo, rwide = 0, 128
                elif part == "hi" and wide == 256:
                    rlo, rwide = 128, 128
                for kk in range(8):
                    nc.tensor.matmul(
                        psA[q][:, off + rlo:off + rlo + rwide],
                        lhsT=curT[c][:, kk * B:(kk + 1) * B],
                        rhs=st[:, kk * wide + rlo:kk * wide + rlo + rwide],
                        start=False,
                        stop=False,
                    )

            # ---- block 0: x @ W_in.T + h @ Wrec(block0).T (+bias) ------
            for kk in range(4):
                nc.tensor.matmul(
                    psA[0][:, 0:128],
                    lhsT=xT_sb[:, kk * B:(kk + 1) * B],
                    rhs=win_sb[:, kk * RP:(kk + 1) * RP],
                    start=False,
                    stop=False,
                )
            rec_blk(0)
            chain_tail(0)

            hid_tile(1)
            rec_blk(1)                  # AG_0 window work
            hid_mms(1)
            chain_tail(1)

            skip_tile(1, 1)
            skip_tile(1, 0)
            hid_tile(2)
            rec_blk(2)                  # AG_1 window work
            skip_one(1, 0)
            skip_one(1, 1, part="lo")   # urgent: block 2's last skip source
            hid_mms(2)
            chain_tail(2)

            skip_tile(1, 2)
            hid_tile(3)
            skip_tile(2, 0)
            rec_blk(3)                  # AG_2 window work
            skip_one(1, 1, part="hi")
            skip_one(1, 2)              # urgent for block 3
            hid_mms(3)
            chain_tail(3)

            rec_blk(4)                  # AG_3 window work
            skip_tile(2, 1)
            hid_tile(4)
            skip_tile(2, 2)
            skip_tile(2, 3)
            skip_one(2, 0)
            skip_one(2, 1)
            skip_one(2, 2)
            skip_one(2, 3, part="lo")   # urgent for block 4
            hid_mms(4)
            chain_tail(4)

            rec_blk(5)                  # AG_4 window work
            hid_tile(5)
            skip_tile(3, 0)
            skip_tile(2, 4)
            skip_one(3, 0)
            skip_one(2, 3, part="hi")
            skip_one(2, 4)              # urgent for block 5
            hid_mms(5)
            chain_tail(5)

            rec_blk(6)                  # AG_5 window work
            skip_tile(3, 1)
            skip_tile(3, 2)
            skip_tile(3, 3)
            skip_tile(3, 4)
            skip_tile(3, 5)
            skip_one(3, 1)
            skip_one(3, 2)
            skip_one(3, 3)
            skip_one(3, 4)
            skip_one(3, 5, part="lo")   # urgent for block 6
            hid_mms(6)
            chain_tail(6)

            skip_tile(3, 6)
            hid_tile(7)
            skip_one(3, 5, part="hi")   # AG_6 window work
            skip_one(3, 6)              # urgent for block 7
            hid_mms(7)

            # block 7 tail: sigmoid -> transpose -> local out matmul -> RS
            cp7 = curp.tile([64, 128], fp32, name="cpart7", tag="cpart", bufs=2)
            nc.scalar.activation(cp7, psA[3][:, 128:256], AF.Sigmoid)
            pt7 = psump.tile([128, B], fp32, name="pt7", tag="pt", bufs=2)
            nc.tensor.transpose(pt7, cp7, ident_sb[:, :])
            cur7T_sb = curp.tile([128, B], fp32, name="cur7T_sb", tag="c7T")
            nc.vector.tensor_copy(cur7T_sb, pt7)

            pso = psump.tile([64, 512], fp32, name="pso", tag="pso")
            nc.tensor.matmul(
                pso, lhsT=ones_sb[:, :], rhs=boutrow_sb[:, :], start=True, stop=False
            )
            nc.tensor.matmul(
                pso, lhsT=cur7T_sb, rhs=wout_sb[:, :], start=False, stop=True
            )
            out_sb = curp.tile([64, 512], fp32, name="out_sb", tag="out_sb")
            nc.vector.tensor_copy(out_sb, pso)
            rs_in = dramp.tile([64, 512], fp32, name="rs_in", tag="rs_in")
            rs_out = dramp.tile([8, 512], fp32, name="rs_out", tag="rs_out")
            nc.scalar.dma_start(out=rs_in, in_=out_sb)
            nc.gpsimd.collective_compute(
                "ReduceScatter",
                mybir.AluOpType.add,
                replica_groups=rg,
                ins=[rs_in.opt()],
                outs=[rs_out.opt()],
            )
            nc.scalar.dma_start(out=t_out[:, :], in_=rs_out)

    nc.compile()
    return nc


def _get_nc():
    global _BUILT
    if _BUILT is None:
        _BUILT = _build()
    return _BUILT


def make_in_maps(x, hidden_states, W_in, b_in, W_hid, b_hid, W_rec, W_skip,
                 W_out, b_out, mask_hid, mask_rec, mask_skip):
    x = np.asarray(x, np.float32)
    h = np.asarray(hidden_states, np.float32)
    W_in = np.asarray(W_in, np.float32)
    b_in = np.asarray(b_in, np.float32)
    W_out = np.asarray(W_out, np.float32)
    b_out = np.asarray(b_out, np.float32)
    Wh = np.asarray(W_hid, np.float32) * np.asarray(mask_hid, np.float32)
    Wr = np.asarray(W_rec, np.float32) * np.asarray(mask_rec, np.float32)
    Ws = np.asarray(W_skip, np.float32) * np.asarray(mask_skip, np.float32)
    b_hid = np.asarray(b_hid, np.float32)

    hT = _pack(h)
    xT = _pack(x)
    ones = np.ones((1, B), np.float32)
    ident = np.eye(B, dtype=np.float32)
    # every core's partial includes the bias and ReduceScatter sums them
    boutrow = np.ascontiguousarray(b_out[None, :]) / NCORES

    in_maps = []
    for c_ in range(NCORES):
        R = slice(c_ * RP, (c_ + 1) * RP)
        biases = [b_in[R]] + [b_hid[i, R] for i in range(7)]
        biasrow = np.zeros((1, 4 * 256), np.float32)
        for j in range(8):
            biasrow[0, j * 128:(j + 1) * 128] = biases[j]
        m = {
            "hT": hT,
            "xT": xT,
            "win": _pack(W_in[R]),
            "hid": np.stack([_pack(Wh[i, R]) for i in range(7)]),
            "biasrow": biasrow,
            "wout": np.ascontiguousarray(W_out[:, R].T),
            "boutrow": boutrow,
            "ones": ones,
            "ident": ident,
        }
        for j in range(7):
            m[f"rec{j}"] = _pack(Wr[j, R])
        packs = {j: _pack(Ws[j - 2, R, :j * W]).reshape(128, j * 8, 128)
                 for j in range(2, 8)}
        for q in range(1, 4):
            a = 2 * q
            for c in range(a + 1):
                if c < a:
                    Pa = packs[a][:, c * 8:(c + 1) * 8, :].reshape(128, 8 * 128)
                    Pb = packs[a + 1][:, c * 8:(c + 1) * 8, :].reshape(128, 8 * 128)
                    m[f"skip{q}_{c}"] = _interleave(
                        np.ascontiguousarray(Pa), np.ascontiguousarray(Pb)
                    )
                else:
                    m[f"skip{q}_{c}"] = np.ascontiguousarray(
                        packs[a + 1][:, c * 8:(c + 1) * 8, :].reshape(128, 8 * 128)
                    )
        in_maps.append(m)
    return in_maps


def run(in_maps, **kw):
    from concourse import bass_utils
    nc = _get_nc()
    return bass_utils.run_bass_kernel_spmd(
        nc, in_maps, core_ids=list(range(NCORES)), **kw
    )


def kernel(**inputs):
    in_maps = make_in_maps(**inputs)
    res = run(in_maps)
    return np.ascontiguousarray(
        np.concatenate([res.results[c]["out"] for c in range(NCORES)], axis=0),
        dtype=np.float32,
    )



# revision 30
# speedup vs baseline: 1.4045x; 1.4045x over previous
"""BrainRNN forward pass on 8 TRN2 NeuronCores (Bass/Tile, SPMD).

Strategy (tensor-parallel over output neurons, low-precision streaming):
  - Each block's 1024 output neurons are row-sharded 128/core; masks are
    folded into weights on the host.  W_rec/W_skip stream as fp8e4m3,
    everything else as bf16 (absmax err ~7e-3 vs 1.7e-2 budget): cuts
    HBM traffic 51.8MB -> ~14MB per core.
  - Matmuls keep activations as the 64-wide stationary operand and the
    streamed weights as the MOVING operand (1 cyc/row sub-fp32 dtypes).
    Chain-independent blocks share one wide matmul via host-side
    M-interleaved packing (rec23 M=256, rec456 M=384, skip quads M=512),
    amortizing LDWEIGHTS + instruction overhead.
  - Skip contributions are regrouped by SOURCE block c (available after
    AllGather_c) and split into an "urgent" solo targeting the next
    sigmoid and a deferred wide group for later AG windows.
  - Chain per block: sigmoid (PSUM->SBUF bf16), scalar bounce of the
    untransposed [64, 128] shard to DRAM, AllGather (bf16, Shared
    output), one HWDGE X-bar transpose-DMA landing directly in curT
    layout (no PE transpose / DVE copy / rearrange unpack).
  - Output block: each core contracts its own 128 rows of cur7 against
    W_out[:, rows].T (N=512); ReduceScatter yields an 8-row batch shard.
"""

import numpy as np
import ml_dtypes

N = 8192
W = 1024
L = 8
B = 64
IN = 512
OUT = 512
NCORES = 8
RP = W // NCORES        # 128 rows per core per block

BF16 = ml_dtypes.bfloat16
FP8 = ml_dtypes.float8_e4m3

# skip groups: (source c, target blocks) -> one packed dram tensor each.
# "urgent" solos feed the next sigmoid; wide groups fill later AG windows.
SKIP_GROUPS = [
    (0, (2, 3)),
    (0, (4, 5, 6, 7)),
    (1, (2,)),
    (1, (3,)),
    (1, (4, 5, 6, 7)),
    (2, (3,)),
    (2, (4, 5, 6, 7)),
    (3, (4,)),
    (3, (5, 6, 7)),
    (4, (5,)),
    (4, (6, 7)),
    (5, (6,)),
    (5, (7,)),
    (6, (7,)),
]

_BUILT = None


def _pack_group(mats, out_dtype):
    """mats: list of [M_i, K] (same K).  Returns [128, (K/128)*sum(M_i)]
    where chunk k holds [A_0[:,k128].T | A_1[:,k128].T | ...]."""
    K = mats[0].shape[1]
    nk = K // 128
    chunks = []
    for k in range(nk):
        cols = [np.asarray(A[:, k * 128:(k + 1) * 128].T, np.float32)
                for A in mats]
        chunks.append(np.concatenate(cols, axis=1))
    return np.ascontiguousarray(
        np.concatenate(chunks, axis=1).astype(out_dtype))


def _pack(A, out_dtype):
    return _pack_group([A], out_dtype)


def _build():
    import concourse.bass as bass
    import concourse.bacc as bacc
    import concourse.mybir as mybir
    import concourse.tile as tile

    fp32 = mybir.dt.float32
    bf16 = mybir.dt.bfloat16
    fp8 = mybir.dt.float8e4
    AF = mybir.ActivationFunctionType

    nc = bacc.Bacc(
        "TRN2",
        target_bir_lowering=False,
        debug=False,
        enable_asserts=False,
        num_devices=NCORES,
    )

    t_hT = nc.dram_tensor("hT", [128, 64 * B], bf16, kind="ExternalInput")
    t_xT = nc.dram_tensor("xT", [128, 4 * B], bf16, kind="ExternalInput")
    t_win = nc.dram_tensor("win", [128, 4 * RP], bf16, kind="ExternalInput")
    t_rec0 = nc.dram_tensor("rec0", [128, 64 * 128], fp8, kind="ExternalInput")
    t_rec1 = nc.dram_tensor("rec1", [128, 64 * 128], fp8, kind="ExternalInput")
    t_rec23 = nc.dram_tensor("rec23", [128, 64 * 256], fp8, kind="ExternalInput")
    t_rec456 = nc.dram_tensor("rec456", [128, 64 * 384], fp8, kind="ExternalInput")
    t_hid = {
        j: nc.dram_tensor(f"hid{j}", [128, 8 * 128], bf16, kind="ExternalInput")
        for j in range(1, 8)
    }
    t_skip = {
        (c, T): nc.dram_tensor(
            f"skip{c}_{''.join(map(str, T))}", [128, 8 * 128 * len(T)], fp8,
            kind="ExternalInput")
        for c, T in SKIP_GROUPS
    }
    t_biasrow = nc.dram_tensor("biasrow", [1, 8 * 128], bf16, kind="ExternalInput")
    t_wout = nc.dram_tensor("wout", [128, 8 * 512], bf16, kind="ExternalInput")
    t_boutrow = nc.dram_tensor("boutrow", [1, 512], bf16, kind="ExternalInput")
    t_ones = nc.dram_tensor("ones", [1, B], bf16, kind="ExternalInput")
    t_ident = nc.dram_tensor("ident", [B, B], bf16, kind="ExternalInput")
    t_out = nc.dram_tensor("out", [64, 512], fp32, kind="ExternalOutput")

    rg = [list(range(NCORES))]

    with tile.TileContext(nc) as tc:
        with (
            tc.tile_pool(name="const", bufs=1) as constp,
            tc.tile_pool(name="w", bufs=1) as wp,
            tc.tile_pool(name="curs", bufs=1) as curp,
            tc.tile_pool(name="psum", bufs=1, space="PSUM") as psump,
            tc.tile_pool(name="dram", bufs=1, space="DRAM") as dramp,
        ):
            # ---- persistent inputs (DMA emission order = priority) -----
            xT_sb = constp.tile([128, 4 * B], bf16, name="xT_sb", tag="xT")
            nc.sync.dma_start(out=xT_sb, in_=t_xT[:, :])
            win_sb = constp.tile([128, 4 * RP], bf16, name="win_sb", tag="win")
            nc.sync.dma_start(out=win_sb, in_=t_win[:, :])
            biasrow_sb = constp.tile([1, 8 * 128], bf16, name="biasrow_sb", tag="br")
            nc.sync.dma_start(out=biasrow_sb, in_=t_biasrow[:, :])
            ones_sb = constp.tile([1, B], bf16, name="ones_sb", tag="ones")
            nc.sync.dma_start(out=ones_sb, in_=t_ones[:, :])
            ident_sb = constp.tile([B, B], bf16, name="ident_sb", tag="ident")
            nc.sync.dma_start(out=ident_sb, in_=t_ident[:, :])
            hT_sb = constp.tile([128, 64 * B], bf16, name="hT_sb", tag="hT")
            for i in range(2):
                nc.sync.dma_start(
                    out=hT_sb[:, i * 2048:(i + 1) * 2048],
                    in_=t_hT[:, i * 2048:(i + 1) * 2048],
                )

            def stream(dst, src, piece_cols):
                total = src.shape[1]
                o = 0
                while o < total:
                    e = min(o + piece_cols, total)
                    nc.sync.dma_start(out=dst[:, o:e], in_=src[:, o:e])
                    o = e

            # weight tiles (persistent, distinct tags)
            rec0_sb = wp.tile([128, 64 * 128], fp8, name="rec0_sb", tag="rec0")
            stream(rec0_sb, t_rec0, 1, 2048)          # 4 x 256KB
            rec1_sb = wp.tile([128, 64 * 128], fp8, name="rec1_sb", tag="rec1")
            stream(rec1_sb, t_rec1, 1, 4096)
            hid_sb = {}
            hid_sb[1] = wp.tile([128, 8 * 128], bf16, name="hid1_sb", tag="hid1")
            nc.sync.dma_start(out=hid_sb[1], in_=t_hid[1][:, :])
            rec23_sb = wp.tile([128, 64 * 256], fp8, name="rec23_sb", tag="rec23")
            stream(rec23_sb, t_rec23, 1, 4096)
            hid_sb[2] = wp.tile([128, 8 * 128], bf16, name="hid2_sb", tag="hid2")
            nc.sync.dma_start(out=hid_sb[2], in_=t_hid[2][:, :])
            skip_sb = {}

            def skip_load(c, T):
                st = wp.tile(
                    [128, 8 * 128 * len(T)], fp8,
                    name=f"skip{c}_{''.join(map(str, T))}_sb",
                    tag=f"sk{c}_{T[0]}",
                )
                stream(st, t_skip[(c, T)], 1, 4096)
                skip_sb[(c, T)] = st

            skip_load(0, (2, 3))
            rec456_sb = wp.tile([128, 64 * 384], fp8, name="rec456_sb", tag="rec456")
            stream(rec456_sb, t_rec456, 1, 4096)
            skip_load(1, (2,))
            hid_sb[3] = wp.tile([128, 8 * 128], bf16, name="hid3_sb", tag="hid3")
            nc.sync.dma_start(out=hid_sb[3], in_=t_hid[3][:, :])
            skip_load(2, (3,))
            skip_load(1, (3,))
            skip_load(0, (4, 5, 6, 7))
            skip_load(1, (4, 5, 6, 7))
            skip_load(2, (4, 5, 6, 7))
            hid_sb[4] = wp.tile([128, 8 * 128], bf16, name="hid4_sb", tag="hid4")
            nc.sync.dma_start(out=hid_sb[4], in_=t_hid[4][:, :])
            skip_load(3, (4,))
            skip_load(3, (5, 6, 7))
            hid_sb[5] = wp.tile([128, 8 * 128], bf16, name="hid5_sb", tag="hid5")
            nc.sync.dma_start(out=hid_sb[5], in_=t_hid[5][:, :])
            skip_load(4, (5,))
            skip_load(4, (6, 7))
            hid_sb[6] = wp.tile([128, 8 * 128], bf16, name="hid6_sb", tag="hid6")
            nc.sync.dma_start(out=hid_sb[6], in_=t_hid[6][:, :])
            skip_load(5, (6,))
            skip_load(5, (7,))
            hid_sb[7] = wp.tile([128, 8 * 128], bf16, name="hid7_sb", tag="hid7")
            nc.sync.dma_start(out=hid_sb[7], in_=t_hid[7][:, :])
            skip_load(6, (7,))
            wout_sb = constp.tile([128, 8 * 512], bf16, name="wout_sb", tag="wout")
            stream(wout_sb, t_wout, 2048)
            boutrow_sb = constp.tile([1, 512], bf16, name="boutrow_sb", tag="bo")
            nc.sync.dma_start(out=boutrow_sb, in_=t_boutrow[:, :])

            # ---- PSUM accumulators ------------------------------------
            psA = psump.tile([64, 512], fp32, name="psA", tag="psA")  # blocks 0-3
            psB = psump.tile([64, 512], fp32, name="psB", tag="psB")  # blocks 4-7

            def pscol(j):
                ps = psA if j < 4 else psB
                o = (j % 4) * 128
                return ps, o

            # open both accumulators with the bias rows
            nc.tensor.matmul(psA, lhsT=ones_sb[:, :], rhs=biasrow_sb[:, 0:512],
                             start=True, stop=False)
            nc.tensor.matmul(psB, lhsT=ones_sb[:, :], rhs=biasrow_sb[:, 512:1024],
                             start=True, stop=False)

            def mm(ps, col, wid, lhsT, rhs, stop=False):
                nc.tensor.matmul(ps[:, col:col + wid], lhsT=lhsT,
                                 rhs=rhs, start=False, stop=stop)

            # x @ W_in.T into block 0
            for kk in range(4):
                mm(psA, 0, 128, xT_sb[:, kk * B:(kk + 1) * B],
                   win_sb[:, kk * RP:(kk + 1) * RP])

            def rec_mms(tile_sb, col, wid, stop_last=False):
                for kg in range(64):
                    mm(psA if col < 512 else psB, col % 512, wid,
                       hT_sb[:, kg * B:(kg + 1) * B],
                       tile_sb[:, kg * wid:(kg + 1) * wid],
                       stop=stop_last and kg == 63)

            curT = [None] * 8

            def hid_mms(j, stop_last=False):
                ps, o = pscol(j)
                for kk in range(8):
                    mm(ps, o, 128, curT[j - 1][:, kk * B:(kk + 1) * B],
                       hid_sb[j][:, kk * 128:(kk + 1) * 128],
                       stop=stop_last and kk == 7)

            def skip_mms(c, T):
                st = skip_sb[(c, T)]
                wid = 128 * len(T)
                ps, o = pscol(T[0])
                for kk in range(8):
                    mm(ps, o, wid, curT[c][:, kk * B:(kk + 1) * B],
                       st[:, kk * wid:(kk + 1) * wid])

            def chain_tail(j):
                """sigmoid -> AllGather (untransposed) -> X-bar transpose.

                The [64, 128] sigmoid output goes straight to the collective;
                the gathered [512, 128] comes back through one HWDGE
                transpose-DMA that lands directly in curT layout.  This
                removes the PE transpose, DVE copy, and rearrange unpack
                from the serial chain."""
                ps, o = pscol(j)
                cp = curp.tile([64, 128], bf16, name=f"cp{j}", tag=f"cp{j}")
                nc.scalar.activation(cp, ps[:, o:o + 128], AF.Sigmoid)
                agin = dramp.tile([64, 128], bf16, name=f"agin{j}", tag=f"agin{j}")
                agout = dramp.tile([8 * 64, 128], bf16, name=f"agout{j}",
                                   tag=f"agout{j}", addr_space="Shared")
                nc.scalar.dma_start(out=agin, in_=cp)
                nc.gpsimd.collective_compute(
                    "AllGather",
                    mybir.AluOpType.bypass,
                    replica_groups=rg,
                    ins=[agin.opt()],
                    outs=[agout.opt()],
                )
                dst = curp.tile([128, 8 * B], bf16, name=f"curT{j}", tag=f"curT{j}")
                nc.scalar.dma_start_transpose(dst[:, :], agout[:, :])
                curT[j] = dst

            def await_round(j):
                pass

            # ---- PE emission order (chain + window back-fill) ----------
            rec_mms(rec0_sb, 0, 128)
            chain_tail(0)

            # AG_0's window is the long one (first-collective setup): all
            # chain-independent rec work back-fills it, keeping the short
            # later windows free of overshooting backfill.
            rec_mms(rec1_sb, 128, 128)
            rec_mms(rec23_sb, 256, 256)
            rec_mms(rec456_sb, 512, 384)
            await_round(0)
            hid_mms(1)                          # urgent (needs AG_0)
            chain_tail(1)

            skip_mms(0, (2, 3))                # AG_1 window (needs AG_0)
            skip_mms(0, (4, 5, 6, 7))
            await_round(1)
            skip_mms(1, (2,))                   # urgent (needs AG_1)
            hid_mms(2)
            chain_tail(2)

            skip_mms(1, (3,))                  # AG_2 window (needs AG_1)
            skip_mms(1, (4, 5, 6, 7))
            await_round(2)
            skip_mms(2, (3,))                   # urgent (needs AG_2)
            hid_mms(3, stop_last=True)          # closes psA
            chain_tail(3)

            skip_mms(2, (4, 5, 6, 7))          # AG_3 window
            await_round(3)
            skip_mms(3, (4,))                   # urgent (needs AG_3)
            hid_mms(4)
            chain_tail(4)

            skip_mms(3, (5, 6, 7))             # AG_4 window
            await_round(4)
            skip_mms(4, (5,))                   # urgent
            hid_mms(5)
            chain_tail(5)

            skip_mms(4, (6, 7))                # AG_5 window
            await_round(5)
            skip_mms(5, (6,))                   # urgent
            hid_mms(6)
            chain_tail(6)

            skip_mms(5, (7,))                  # AG_6 window
            await_round(6)
            skip_mms(6, (7,))                   # urgent
            hid_mms(7, stop_last=True)          # closes psB

            # ---- block 7 tail: AllGather cur7 like every other block and
            # compute the full [64, 512] output redundantly on each core.
            # The AG fires right after sigmoid_7 (no psum serialization) and
            # an AG of 16KB beats a ReduceScatter of 128KB by ~5us.
            chain_tail(7)
            pso = psump.tile([64, 512], fp32, name="pso", tag="pso")
            nc.tensor.matmul(
                pso, lhsT=ones_sb[:, :], rhs=boutrow_sb[:, :], start=True,
                stop=False)
            for kk in range(8):
                nc.tensor.matmul(
                    pso, lhsT=curT[7][:, kk * B:(kk + 1) * B],
                    rhs=wout_sb[:, kk * 512:(kk + 1) * 512],
                    start=False, stop=(kk == 7))
            out_sb = curp.tile([64, 512], fp32, name="out_sb", tag="out_sb")
            nc.vector.tensor_copy(out_sb, pso)
            nc.scalar.dma_start(out=t_out[:, :], in_=out_sb)

    nc.compile()
    return nc


def _get_nc():
    global _BUILT
    if _BUILT is None:
        _BUILT = _build()
    return _BUILT


def make_in_maps(x, hidden_states, W_in, b_in, W_hid, b_hid, W_rec, W_skip,
                 W_out, b_out, mask_hid, mask_rec, mask_skip):
    x = np.asarray(x, np.float32)
    h = np.asarray(hidden_states, np.float32)
    W_in = np.asarray(W_in, np.float32)
    b_in = np.asarray(b_in, np.float32)
    W_out = np.asarray(W_out, np.float32)
    b_out = np.asarray(b_out, np.float32)
    Wh = np.asarray(W_hid, np.float32) * np.asarray(mask_hid, np.float32)
    Wr = np.asarray(W_rec, np.float32) * np.asarray(mask_rec, np.float32)
    Ws = np.asarray(W_skip, np.float32) * np.asarray(mask_skip, np.float32)
    b_hid = np.asarray(b_hid, np.float32)

    hT = _pack(h, BF16)
    xT = _pack(x, BF16)
    ones = np.ones((1, B), BF16)
    ident = np.eye(B).astype(BF16)
    boutrow = np.ascontiguousarray(b_out[None, :]).astype(BF16)

    in_maps = []
    for c_ in range(NCORES):
        R = slice(c_ * RP, (c_ + 1) * RP)
        biases = [b_in[R]] + [b_hid[i, R] for i in range(7)]
        biasrow = np.zeros((1, 8 * 128), np.float32)
        for j in range(8):
            biasrow[0, j * 128:(j + 1) * 128] = biases[j]
        m = {
            "hT": hT,
            "xT": xT,
            "win": _pack(W_in[R], BF16),
            "rec0": _pack(Wr[0, R], FP8),
            "rec1": _pack(Wr[1, R], FP8),
            "rec23": _pack_group([Wr[2, R], Wr[3, R]], FP8),
            "rec456": _pack_group([Wr[4, R], Wr[5, R], Wr[6, R]], FP8),
            "biasrow": biasrow.astype(BF16),
            "wout": _pack(W_out, BF16),
            "boutrow": boutrow,
            "ones": ones,
            "ident": ident,
        }
        for j in range(1, 8):
            m[f"hid{j}"] = _pack(Wh[j - 1, R], BF16)
        for c, T in SKIP_GROUPS:
            mats = [Ws[t - 2, R, c * W:(c + 1) * W] for t in T]
            m[f"skip{c}_{''.join(map(str, T))}"] = _pack_group(mats, FP8)
        in_maps.append(m)
    return in_maps


def run(in_maps, **kw):
    from concourse import bass_utils
    nc = _get_nc()
    return bass_utils.run_bass_kernel_spmd(
        nc, in_maps, core_ids=list(range(NCORES)), **kw
    )


def kernel(**inputs):
    in_maps = make_in_maps(**inputs)
    res = run(in_maps)
    return np.ascontiguousarray(res.results[0]["out"], dtype=np.float32)


# revision 31
# speedup vs baseline: 1.5707x; 1.1184x over previous
"""BrainRNN forward pass on 8 TRN2 NeuronCores (Bass/Tile, SPMD).

Strategy (tensor-parallel over output neurons, low-precision streaming):
  - Each block's 1024 output neurons are row-sharded 128/core; masks are
    folded into weights on the host.  W_rec/W_skip stream as fp8e4m3,
    everything else as bf16 (absmax err ~7e-3 vs 1.7e-2 budget): cuts
    HBM traffic 51.8MB -> ~14MB per core.
  - Matmuls keep activations as the 64-wide stationary operand and the
    streamed weights as the MOVING operand (1 cyc/row sub-fp32 dtypes).
    Chain-independent blocks share one wide matmul via host-side
    M-interleaved packing (rec23 M=256, rec456 M=384, skip quads M=512),
    amortizing LDWEIGHTS + instruction overhead.
  - Skip contributions are regrouped by SOURCE block c (available after
    AllGather_c) and split into an "urgent" solo targeting the next
    sigmoid and a deferred wide group for later AG windows.
  - Chain per block: sigmoid (PSUM->SBUF bf16), scalar bounce of the
    untransposed [64, 128] shard to DRAM, AllGather (bf16, Shared
    output), one HWDGE X-bar transpose-DMA landing directly in curT
    layout (no PE transpose / DVE copy / rearrange unpack).
  - Output block: each core contracts its own 128 rows of cur7 against
    W_out[:, rows].T (N=512); ReduceScatter yields an 8-row batch shard.
"""

import numpy as np
import ml_dtypes

N = 8192
W = 1024
L = 8
B = 64
IN = 512
OUT = 512
NCORES = 8
RP = W // NCORES        # 128 rows per core per block

BF16 = ml_dtypes.bfloat16
FP8 = ml_dtypes.float8_e4m3

# skip groups: (source c, target blocks) -> one packed dram tensor each.
# "urgent" solos feed the next sigmoid; wide groups fill later AG windows.
SKIP_GROUPS = [
    (0, (2, 3)),
    (0, (4, 5, 6, 7)),
    (1, (2,)),
    (1, (3,)),
    (1, (4, 5, 6, 7)),
    (2, (3,)),
    (2, (4, 5, 6, 7)),
    (3, (4,)),
    (3, (5, 6, 7)),
    (4, (5,)),
    (4, (6, 7)),
    (5, (6,)),
    (5, (7,)),
    (6, (7,)),
]

_BUILT = None


def _pack_group(mats, out_dtype):
    """mats: list of [M_i, K] (same K).  Returns [128, (K/128)*sum(M_i)]
    where chunk k holds [A_0[:,k128].T | A_1[:,k128].T | ...]."""
    K = mats[0].shape[1]
    nk = K // 128
    chunks = []
    for k in range(nk):
        cols = [np.asarray(A[:, k * 128:(k + 1) * 128].T, np.float32)
                for A in mats]
        chunks.append(np.concatenate(cols, axis=1))
    return np.ascontiguousarray(
        np.concatenate(chunks, axis=1).astype(out_dtype))


def _pack(A, out_dtype):
    return _pack_group([A], out_dtype)


def _build():
    import concourse.bass as bass
    import concourse.bacc as bacc
    import concourse.mybir as mybir
    import concourse.tile as tile

    fp32 = mybir.dt.float32
    bf16 = mybir.dt.bfloat16
    fp8 = mybir.dt.float8e4
    AF = mybir.ActivationFunctionType

    nc = bacc.Bacc(
        "TRN2",
        target_bir_lowering=False,
        debug=False,
        enable_asserts=False,
        num_devices=NCORES,
    )

    t_hT = nc.dram_tensor("hT", [128, 64 * B], bf16, kind="ExternalInput")
    t_xT = nc.dram_tensor("xT", [128, 4 * B], bf16, kind="ExternalInput")
    t_win = nc.dram_tensor("win", [128, 4 * RP], bf16, kind="ExternalInput")
    t_rec0 = nc.dram_tensor("rec0", [128, 64 * 128], fp8, kind="ExternalInput")
    t_rec1 = nc.dram_tensor("rec1", [128, 64 * 128], fp8, kind="ExternalInput")
    t_rec23 = nc.dram_tensor("rec23", [128, 64 * 256], fp8, kind="ExternalInput")
    t_rec456 = nc.dram_tensor("rec456", [128, 64 * 384], fp8, kind="ExternalInput")
    t_hid = {
        j: nc.dram_tensor(f"hid{j}", [128, 8 * 128], bf16, kind="ExternalInput")
        for j in range(1, 8)
    }
    t_skip = {
        (c, T): nc.dram_tensor(
            f"skip{c}_{''.join(map(str, T))}", [128, 8 * 128 * len(T)], fp8,
            kind="ExternalInput")
        for c, T in SKIP_GROUPS
    }
    t_biasrow = nc.dram_tensor("biasrow", [1, 8 * 128], bf16, kind="ExternalInput")
    t_wout = nc.dram_tensor("wout", [128, 8 * 512], bf16, kind="ExternalInput")
    t_boutrow = nc.dram_tensor("boutrow", [1, 512], bf16, kind="ExternalInput")
    t_ones = nc.dram_tensor("ones", [1, B], bf16, kind="ExternalInput")
    t_ident = nc.dram_tensor("ident", [B, B], bf16, kind="ExternalInput")
    t_out = nc.dram_tensor("out", [64, 512], fp32, kind="ExternalOutput")

    rg = [list(range(NCORES))]

    with tile.TileContext(nc) as tc:
        with (
            tc.tile_pool(name="const", bufs=1) as constp,
            tc.tile_pool(name="w", bufs=1) as wp,
            tc.tile_pool(name="curs", bufs=1) as curp,
            tc.tile_pool(name="psum", bufs=1, space="PSUM") as psump,
            tc.tile_pool(name="dram", bufs=1, space="DRAM") as dramp,
        ):
            # ---- persistent inputs (DMA emission order = priority) -----
            xT_sb = constp.tile([128, 4 * B], bf16, name="xT_sb", tag="xT")
            nc.sync.dma_start(out=xT_sb, in_=t_xT[:, :])
            win_sb = constp.tile([128, 4 * RP], bf16, name="win_sb", tag="win")
            nc.sync.dma_start(out=win_sb, in_=t_win[:, :])
            biasrow_sb = constp.tile([1, 8 * 128], bf16, name="biasrow_sb", tag="br")
            nc.sync.dma_start(out=biasrow_sb, in_=t_biasrow[:, :])
            ones_sb = constp.tile([1, B], bf16, name="ones_sb", tag="ones")
            nc.sync.dma_start(out=ones_sb, in_=t_ones[:, :])
            ident_sb = constp.tile([B, B], bf16, name="ident_sb", tag="ident")
            nc.sync.dma_start(out=ident_sb, in_=t_ident[:, :])
            hT_sb = constp.tile([128, 64 * B], bf16, name="hT_sb", tag="hT")
            for i in range(2):
                nc.sync.dma_start(
                    out=hT_sb[:, i * 2048:(i + 1) * 2048],
                    in_=t_hT[:, i * 2048:(i + 1) * 2048],
                )

            def stream(dst, src, piece_cols):
                total = src.shape[1]
                o = 0
                while o < total:
                    e = min(o + piece_cols, total)
                    nc.sync.dma_start(out=dst[:, o:e], in_=src[:, o:e])
                    o = e

            # weight tiles (persistent, distinct tags)
            rec0_sb = wp.tile([128, 64 * 128], fp8, name="rec0_sb", tag="rec0")
            stream(rec0_sb, t_rec0, 1, 2048)          # 4 x 256KB
            rec1_sb = wp.tile([128, 64 * 128], fp8, name="rec1_sb", tag="rec1")
            stream(rec1_sb, t_rec1, 1, 4096)
            hid_sb = {}
            hid_sb[1] = wp.tile([128, 8 * 128], bf16, name="hid1_sb", tag="hid1")
            nc.sync.dma_start(out=hid_sb[1], in_=t_hid[1][:, :])
            rec23_sb = wp.tile([128, 64 * 256], fp8, name="rec23_sb", tag="rec23")
            stream(rec23_sb, t_rec23, 1, 4096)
            hid_sb[2] = wp.tile([128, 8 * 128], bf16, name="hid2_sb", tag="hid2")
            nc.sync.dma_start(out=hid_sb[2], in_=t_hid[2][:, :])
            skip_sb = {}

            def skip_load(c, T):
                st = wp.tile(
                    [128, 8 * 128 * len(T)], fp8,
                    name=f"skip{c}_{''.join(map(str, T))}_sb",
                    tag=f"sk{c}_{T[0]}",
                )
                stream(st, t_skip[(c, T)], 1, 4096)
                skip_sb[(c, T)] = st

            skip_load(0, (2, 3))
            rec456_sb = wp.tile([128, 64 * 384], fp8, name="rec456_sb", tag="rec456")
            stream(rec456_sb, t_rec456, 1, 4096)
            skip_load(1, (2,))
            hid_sb[3] = wp.tile([128, 8 * 128], bf16, name="hid3_sb", tag="hid3")
            nc.sync.dma_start(out=hid_sb[3], in_=t_hid[3][:, :])
            skip_load(2, (3,))
            skip_load(1, (3,))
            skip_load(0, (4, 5, 6, 7))
            skip_load(1, (4, 5, 6, 7))
            skip_load(2, (4, 5, 6, 7))
            hid_sb[4] = wp.tile([128, 8 * 128], bf16, name="hid4_sb", tag="hid4")
            nc.sync.dma_start(out=hid_sb[4], in_=t_hid[4][:, :])
            skip_load(3, (4,))
            skip_load(3, (5, 6, 7))
            hid_sb[5] = wp.tile([128, 8 * 128], bf16, name="hid5_sb", tag="hid5")
            nc.sync.dma_start(out=hid_sb[5], in_=t_hid[5][:, :])
            skip_load(4, (5,))
            skip_load(4, (6, 7))
            hid_sb[6] = wp.tile([128, 8 * 128], bf16, name="hid6_sb", tag="hid6")
            nc.sync.dma_start(out=hid_sb[6], in_=t_hid[6][:, :])
            skip_load(5, (6,))
            skip_load(5, (7,))
            hid_sb[7] = wp.tile([128, 8 * 128], bf16, name="hid7_sb", tag="hid7")
            nc.sync.dma_start(out=hid_sb[7], in_=t_hid[7][:, :])
            skip_load(6, (7,))
            wout_sb = constp.tile([128, 8 * 512], bf16, name="wout_sb", tag="wout")
            stream(wout_sb, t_wout, 2048)
            boutrow_sb = constp.tile([1, 512], bf16, name="boutrow_sb", tag="bo")
            nc.sync.dma_start(out=boutrow_sb, in_=t_boutrow[:, :])

            # ---- PSUM accumulators ------------------------------------
            psA = psump.tile([64, 512], fp32, name="psA", tag="psA")  # blocks 0-3
            psB = psump.tile([64, 512], fp32, name="psB", tag="psB")  # blocks 4-7

            def pscol(j):
                ps = psA if j < 4 else psB
                o = (j % 4) * 128
                return ps, o

            # open both accumulators with the bias rows
            nc.tensor.matmul(psA, lhsT=ones_sb[:, :], rhs=biasrow_sb[:, 0:512],
                             start=True, stop=False)
            nc.tensor.matmul(psB, lhsT=ones_sb[:, :], rhs=biasrow_sb[:, 512:1024],
                             start=True, stop=False)

            def mm(ps, col, wid, lhsT, rhs, stop=False):
                nc.tensor.matmul(ps[:, col:col + wid], lhsT=lhsT,
                                 rhs=rhs, start=False, stop=stop)

            # x @ W_in.T into block 0
            for kk in range(4):
                mm(psA, 0, 128, xT_sb[:, kk * B:(kk + 1) * B],
                   win_sb[:, kk * RP:(kk + 1) * RP])

            def rec_mms(tile_sb, col, wid, stop_last=False):
                for kg in range(64):
                    mm(psA if col < 512 else psB, col % 512, wid,
                       hT_sb[:, kg * B:(kg + 1) * B],
                       tile_sb[:, kg * wid:(kg + 1) * wid],
                       stop=stop_last and kg == 63)

            curT = [None] * 8

            def hid_mms(j, stop_last=False):
                ps, o = pscol(j)
                for kk in range(8):
                    mm(ps, o, 128, curT[j - 1][:, kk * B:(kk + 1) * B],
                       hid_sb[j][:, kk * 128:(kk + 1) * 128],
                       stop=stop_last and kk == 7)

            def skip_mms(c, T):
                st = skip_sb[(c, T)]
                wid = 128 * len(T)
                ps, o = pscol(T[0])
                for kk in range(8):
                    mm(ps, o, wid, curT[c][:, kk * B:(kk + 1) * B],
                       st[:, kk * wid:(kk + 1) * wid])

            def chain_tail(j):
                """sigmoid -> AllGather (untransposed) -> X-bar transpose.

                The [64, 128] sigmoid output goes straight to the collective;
                the gathered [512, 128] comes back through one HWDGE
                transpose-DMA that lands directly in curT layout.  This
                removes the PE transpose, DVE copy, and rearrange unpack
                from the serial chain."""
                ps, o = pscol(j)
                cp = curp.tile([64, 128], bf16, name=f"cp{j}", tag=f"cp{j}")
                nc.scalar.activation(cp, ps[:, o:o + 128], AF.Sigmoid)
                agin = dramp.tile([64, 128], bf16, name=f"agin{j}", tag=f"agin{j}")
                agout = dramp.tile([8 * 64, 128], bf16, name=f"agout{j}",
                                   tag=f"agout{j}", addr_space="Shared")
                nc.scalar.dma_start(out=agin, in_=cp)
                nc.gpsimd.collective_compute(
                    "AllGather",
                    mybir.AluOpType.bypass,
                    replica_groups=rg,
                    ins=[agin.opt()],
                    outs=[agout.opt()],
                )
                dst = curp.tile([128, 8 * B], bf16, name=f"curT{j}", tag=f"curT{j}")
                nc.scalar.dma_start_transpose(dst[:, :], agout[:, :])
                curT[j] = dst

            def await_round(j):
                pass

            # ---- PE emission order (chain + window back-fill) ----------
            rec_mms(rec0_sb, 0, 128)
            chain_tail(0)

            rec_mms(rec1_sb, 128, 128)         # AG_0 window
            await_round(0)
            hid_mms(1)                          # urgent (needs AG_0)
            chain_tail(1)

            rec_mms(rec23_sb, 256, 256)        # AG_1 window
            skip_mms(0, (2, 3))
            await_round(1)
            skip_mms(1, (2,))                   # urgent (needs AG_1)
            hid_mms(2)
            chain_tail(2)

            rec_mms(rec456_sb, 512, 384)       # AG_2 window
            skip_mms(1, (3,))
            skip_mms(0, (4, 5, 6, 7))
            skip_mms(1, (4, 5, 6, 7))
            await_round(2)
            skip_mms(2, (3,))                   # urgent (needs AG_2)
            hid_mms(3, stop_last=True)          # closes psA
            chain_tail(3)

            skip_mms(2, (4, 5, 6, 7))          # AG_3 window
            await_round(3)
            skip_mms(3, (4,))                   # urgent (needs AG_3)
            hid_mms(4)
            chain_tail(4)

            skip_mms(3, (5, 6, 7))             # AG_4 window
            await_round(4)
            skip_mms(4, (5,))                   # urgent
            hid_mms(5)
            chain_tail(5)

            skip_mms(4, (6, 7))                # AG_5 window
            await_round(5)
            skip_mms(5, (6,))                   # urgent
            hid_mms(6)
            chain_tail(6)

            skip_mms(5, (7,))                  # AG_6 window
            await_round(6)
            skip_mms(6, (7,))                   # urgent
            hid_mms(7, stop_last=True)          # closes psB

            # ---- block 7 tail: AllGather cur7 like every other block and
            # compute the full [64, 512] output redundantly on each core.
            # The AG fires right after sigmoid_7 (no psum serialization) and
            # an AG of 16KB beats a ReduceScatter of 128KB by ~5us.
            chain_tail(7)
            pso = psump.tile([64, 512], fp32, name="pso", tag="pso")
            nc.tensor.matmul(
                pso, lhsT=ones_sb[:, :], rhs=boutrow_sb[:, :], start=True,
                stop=False)
            for kk in range(8):
                nc.tensor.matmul(
                    pso, lhsT=curT[7][:, kk * B:(kk + 1) * B],
                    rhs=wout_sb[:, kk * 512:(kk + 1) * 512],
                    start=False, stop=(kk == 7))
            out_sb = curp.tile([64, 512], fp32, name="out_sb", tag="out_sb")
            nc.vector.tensor_copy(out_sb, pso)
            nc.scalar.dma_start(out=t_out[:, :], in_=out_sb)

    nc.compile()
    return nc


def _get_nc():
    global _BUILT
    if _BUILT is None:
        _BUILT = _build()
    return _BUILT


def make_in_maps(x, hidden_states, W_in, b_in, W_hid, b_hid, W_rec, W_skip,
                 W_out, b_out, mask_hid, mask_rec, mask_skip):
    x = np.asarray(x, np.float32)
    h = np.asarray(hidden_states, np.float32)
    W_in = np.asarray(W_in, np.float32)
    b_in = np.asarray(b_in, np.float32)
    W_out = np.asarray(W_out, np.float32)
    b_out = np.asarray(b_out, np.float32)
    Wh = np.asarray(W_hid, np.float32) * np.asarray(mask_hid, np.float32)
    Wr = np.asarray(W_rec, np.float32) * np.asarray(mask_rec, np.float32)
    Ws = np.asarray(W_skip, np.float32) * np.asarray(mask_skip, np.float32)
    b_hid = np.asarray(b_hid, np.float32)

    hT = _pack(h, BF16)
    xT = _pack(x, BF16)
    ones = np.ones((1, B), BF16)
    ident = np.eye(B).astype(BF16)
    boutrow = np.ascontiguousarray(b_out[None, :]).astype(BF16)

    in_maps = []
    for c_ in range(NCORES):
        R = slice(c_ * RP, (c_ + 1) * RP)
        biases = [b_in[R]] + [b_hid[i, R] for i in range(7)]
        biasrow = np.zeros((1, 8 * 128), np.float32)
        for j in range(8):
            biasrow[0, j * 128:(j + 1) * 128] = biases[j]
        m = {
            "hT": hT,
            "xT": xT,
            "win": _pack(W_in[R], BF16),
            "rec0": _pack(Wr[0, R], FP8),
            "rec1": _pack(Wr[1, R], FP8),
            "rec23": _pack_group([Wr[2, R], Wr[3, R]], FP8),
            "rec456": _pack_group([Wr[4, R], Wr[5, R], Wr[6, R]], FP8),
            "biasrow": biasrow.astype(BF16),
            "wout": _pack(W_out, BF16),
            "boutrow": boutrow,
            "ones": ones,
            "ident": ident,
        }
        for j in range(1, 8):
            m[f"hid{j}"] = _pack(Wh[j - 1, R], BF16)
        for c, T in SKIP_GROUPS:
            mats = [Ws[t - 2, R, c * W:(c + 1) * W] for t in T]
            m[f"skip{c}_{''.join(map(str, T))}"] = _pack_group(mats, FP8)
        in_maps.append(m)
    return in_maps


def run(in_maps, **kw):
    from concourse import bass_utils
    nc = _get_nc()
    return bass_utils.run_bass_kernel_spmd(
        nc, in_maps, core_ids=list(range(NCORES)), **kw
    )


def kernel(**inputs):
    in_maps = make_in_maps(**inputs)
    res = run(in_maps)
    return np.ascontiguousarray(res.results[0]["out"], dtype=np.float32)
